# revision 11
# baseline (speedup 1.0000x reference)
"""MoE gating kernel (logits = x @ W^T + noise; softmax; top-2) on 8 trn2 cores.

Sharding: data-parallel over tokens. Each core gets 2048 of the 16384 tokens,
the full (64, 4096) router weight, and its noise slice. No collectives.

Per-core dataflow (2048 tokens, 4 groups of 512):
  - DMA x naturally as [128 tok, H] tiles.
  - PE-transpose 128x128 blocks (fp32, exact) into 2-bank PSUM strips,
    copy to SBUF as float32r (the copy is the fp32r rounding producer).
  - logitsT accumulates in PSUM over 32 k-tiles with float32r matmuls
    (full rate; E=64 so two k-tiles run concurrently via column tiling:
    tile_position (0,0)/(0,64) with outputs in partition halves).
  - PE-transpose logits back to [128 tok, 64 E] (the two halves accumulate),
    add noise (DVE).
  - Top-2 via DVE Max8 + MaxIndex; softmax denominator via ACT exp with
    accumulate; weights = exp(top_i - max) / sum. The reference's (XLA CPU)
    vectorized exp returns exactly 0 below EXP_ZERO_CUT, which collapses its
    2nd weight to 0 and makes top_k tie-break to the lowest index; we
    replicate that explicitly.
"""

import numpy as np

import concourse.bacc as bacc
import concourse.mybir as mybir
import concourse.tile as tile
from concourse.masks import make_identity

F32 = mybir.dt.float32
F32R = mybir.dt.float32r
I32 = mybir.dt.int32
U32 = mybir.dt.uint32
AF = mybir.ActivationFunctionType
ALU = mybir.AluOpType

NCORES = 8
N_TOTAL = 16384
NT = N_TOTAL // NCORES  # 2048 tokens per core
H = 4096
E = 64
TOPK = 2
G = 4  # token groups per core
GT = NT // G  # 512 tokens per group
KT = H // 128  # 32 contraction tiles
NSUB = NT // 128  # 16 token subtiles of 128 per core

# k-pairs the col-tiled matmul trails behind the transposes, giving the
# PSUM->SBUF copy engines time to land xT before PE needs it.
MM_LAG = 2

# Largest float32 d for which XLA's CPU *vectorized* exp(d) == 0.0 (its
# gradual-underflow path bottoms out around 5.6e-43; measured by bisection
# against jax.numpy.exp over arrays on CPU).
EXP_ZERO_CUT = -97.28622


def build_module(reps=1, act_copy_mod=3, hw_loop=False):
    nc = bacc.Bacc("TRN2")

    x = nc.dram_tensor("x", [NT, H], F32, kind="ExternalInput")
    w = nc.dram_tensor("w", [E, H], F32, kind="ExternalInput")
    noise = nc.dram_tensor("noise", [NT, E], F32, kind="ExternalInput")
    out_idx = nc.dram_tensor("out_idx", [NT, TOPK], I32, kind="ExternalOutput")
    out_wt = nc.dram_tensor("out_wt", [NT, TOPK], F32, kind="ExternalOutput")

    with tile.TileContext(nc) as tc:
        with (
            tc.tile_pool(name="consts", bufs=1) as consts,
            tc.tile_pool(name="wt_keep", bufs=1) as wt_keep,
            tc.tile_pool(name="res", bufs=1) as res_pool,
        ):
            ident = consts.tile([128, 128], F32)
            make_identity(nc, ident[:])

            # ---- W^T prep: [64, H] -> 32 tiles of [128 h, 64 e] ----
            wt_sb = wt_keep.tile([128, KT * E], F32R)  # 8KB/partition
            with (
                tc.tile_pool(name="wprep", bufs=1) as wprep,
                tc.tile_pool(name="wt_ps_pool", bufs=2, space="PSUM") as wt_ps_pool,
            ):
                w_sb = wprep.tile([E, H], F32)
                nc.sync.dma_start(w_sb[:], w[:])
                for r in range(KT // 8):
                    wt_ps = wt_ps_pool.tile([128, 512], F32)
                    for q in range(8):
                        k = r * 8 + q
                        nc.tensor.transpose(
                            wt_ps[:, q * E : (q + 1) * E],
                            w_sb[:, k * 128 : (k + 1) * 128],
                            ident[0:E, 0:E],
                        )
                    nc.vector.tensor_copy(wt_sb[:, r * 512 : (r + 1) * 512], wt_ps[:])
            wt3 = wt_sb[:].rearrange("p (k e) -> p k e", e=E)

            # ---- per-core result accumulators ----
            vals8_t = res_pool.tile([128, NSUB * 8], F32)
            idx8_t = res_pool.tile([128, NSUB * 8], U32)
            negm = res_pool.tile([128, NSUB], F32)
            ssum = res_pool.tile([128, NSUB], F32)
            v8 = vals8_t[:].rearrange("p (s k) -> p s k", k=8)
            i8 = idx8_t[:].rearrange("p (s k) -> p s k", k=8)

            with (
                tc.tile_pool(name="xin", bufs=12) as xin_pool,
                tc.tile_pool(name="xt_ps_pool", bufs=2, space="PSUM") as xt_ps_pool,
                tc.tile_pool(name="xt_sb_pool", bufs=4) as xt_sb_pool,
                tc.tile_pool(name="lg_ps_pool", bufs=2, space="PSUM") as lg_ps_pool,
                tc.tile_pool(name="lg_sb_pool", bufs=2) as lg_sb_pool,
                tc.tile_pool(name="l2_ps_pool", bufs=2, space="PSUM") as l2_ps_pool,
                tc.tile_pool(name="small", bufs=2) as small_pool,
            ):
                copy_ctr = 0

                import contextlib

                if hw_loop:
                    rep_ctx = lambda: tc.For_i(0, reps, 1)
                    rep_iter = [0]
                else:
                    rep_ctx = contextlib.nullcontext
                    rep_iter = range(reps)
                with rep_ctx() as _loop:
                  for _rep in rep_iter:
                    for g in range(G):
                        # x tiles: 4 partition-tiles x 2 column halves (1MB DMAs)
                        xg = []
                        for j in range(4):
                            row0 = g * GT + j * 128
                            halves = []
                            for hh in range(2):
                                xt_in = xin_pool.tile([128, H // 2], F32)
                                nc.sync.dma_start(
                                    xt_in[:],
                                    x[
                                        row0 : row0 + 128,
                                        hh * (H // 2) : (hh + 1) * (H // 2),
                                    ],
                                )
                                halves.append(xt_in)
                            xg.append(halves)

                        noise_sb = small_pool.tile([128, 4 * E], F32)
                        nc.sync.dma_start(
                            noise_sb[:].rearrange("p (j e) -> p j e", e=E),
                            noise[g * GT : (g + 1) * GT, :].rearrange(
                                "(j p) e -> p j e", p=128
                            ),
                        )

                        lg_ps = lg_ps_pool.tile([E, GT], F32)
                        pend = []
                        for kp in range(KT // 2):  # k-pairs
                            xt_ps = xt_ps_pool.tile([128, 2 * GT], F32)  # 2 banks
                            for half in range(2):
                                k = 2 * kp + half
                                hh, kk = divmod(k, KT // 2)
                                for j in range(4):
                                    nc.tensor.transpose(
                                        xt_ps[
                                            :,
                                            half * GT
                                            + j * 128 : half * GT
                                            + (j + 1) * 128,
                                        ],
                                        xg[j][hh][:, kk * 128 : (kk + 1) * 128],
                                        ident[:],
                                    )
                            xt_sbt = xt_sb_pool.tile([128, 2 * GT], F32R)
                            if copy_ctr % 7 < act_copy_mod:
                                nc.vector.tensor_copy(xt_sbt[:], xt_ps[:])
                            else:
                                nc.scalar.copy(xt_sbt[:], xt_ps[:])
                            copy_ctr += 1
                            pend.append((kp, xt_sbt))
                            if len(pend) > MM_LAG:
                                mkp, mt = pend.pop(0)
                                for half in range(2):
                                    mk = 2 * mkp + half
                                    nc.tensor.matmul(
                                        lg_ps[:],
                                        wt3[:, mk, :],
                                        mt[:, half * GT : (half + 1) * GT],
                                        start=(mk == 0),
                                        stop=(mk == KT - 1),
                                        skip_group_check=True,
                                    )
                        for mkp, mt in pend:
                            for half in range(2):
                                mk = 2 * mkp + half
                                nc.tensor.matmul(
                                    lg_ps[:],
                                    wt3[:, mk, :],
                                    mt[:, half * GT : (half + 1) * GT],
                                    start=(mk == 0),
                                    stop=(mk == KT - 1),
                                    skip_group_check=True,
                                )

                        # logits [64, 512] -> [128 tok, 4x64], then + noise
                        lg_sb = lg_sb_pool.tile([E, GT], F32)
                        nc.vector.tensor_copy(lg_sb[:], lg_ps[:])
                        l2_ps = l2_ps_pool.tile([128, 4 * E], F32)
                        for j in range(4):
                            nc.tensor.transpose(
                                l2_ps[:, j * E : (j + 1) * E],
                                lg_sb[:, j * 128 : (j + 1) * 128],
                                ident[0:E, 0:E],
                            )
                        z_sb = small_pool.tile([128, 4 * E], F32)
                        nc.vector.tensor_add(z_sb[:], l2_ps[:], noise_sb[:])
                        p_scr = small_pool.tile([128, 4 * E], F32)
                        z3 = z_sb[:].rearrange("p (j e) -> p j e", e=E)
                        p3 = p_scr[:].rearrange("p (j e) -> p j e", e=E)

                        for j in range(4):
                            s = g * 4 + j
                            nc.vector.max(v8[:, s, :], z3[:, j, :])
                            nc.vector.max_index(i8[:, s, :], v8[:, s, :], z3[:, j, :])
                        nc.vector.tensor_scalar(
                            negm[:, g * 4 : (g + 1) * 4],
                            v8[:, g * 4 : (g + 1) * 4, 0],
                            -1.0,
                            None,
                            ALU.mult,
                        )
                        for j in range(4):
                            s = g * 4 + j
                            nc.scalar.activation(
                                p3[:, j, :],
                                z3[:, j, :],
                                AF.Exp,
                                bias=negm[:, s : s + 1],
                                scale=1.0,
                                accum_out=ssum[:, s : s + 1],
                            )

                # ---- tail: weights + outputs ----
                d2 = res_pool.tile([128, NSUB], F32)
                nc.vector.tensor_sub(d2[:], v8[:, :, 1], v8[:, :, 0])
                e2 = res_pool.tile([128, NSUB], F32)
                nc.scalar.activation(e2[:], d2[:], AF.Exp)
                rall = res_pool.tile([128, NSUB], F32)
                nc.vector.reciprocal(rall[:], ssum[:])
                w1 = res_pool.tile([128, NSUB], F32)
                nc.vector.tensor_mul(w1[:], e2[:], rall[:])

                # Reference semantics: when d2 <= EXP_ZERO_CUT the reference's
                # 2nd softmax weight is exactly 0 and top_k tie-breaks to the
                # lowest index among the zero scores: 0, or 1 if argmax is 0.
                cond = res_pool.tile([128, NSUB], U32)
                nc.vector.tensor_scalar(cond[:], d2[:], EXP_ZERO_CUT, None, ALU.is_le)
                zeros = res_pool.tile([128, NSUB], F32)
                nc.vector.memset(zeros[:], 0.0)
                nc.vector.copy_predicated(w1[:], cond[:], zeros[:])
                idx1f = res_pool.tile([128, NSUB], F32)
                nc.vector.tensor_copy(idx1f[:], i8[:, :, 1])
                idx0f = res_pool.tile([128, NSUB], F32)
                nc.vector.tensor_copy(idx0f[:], i8[:, :, 0])
                alt = res_pool.tile([128, NSUB], F32)
                nc.vector.tensor_scalar(alt[:], idx0f[:], 0.0, None, ALU.is_equal)
                nc.vector.copy_predicated(idx1f[:], cond[:], alt[:])

                wout = res_pool.tile([128, NSUB * 2], F32)
                wo3 = wout[:].rearrange("p (s k) -> p s k", k=2)
                nc.vector.tensor_copy(wo3[:, :, 0], rall[:])
                nc.vector.tensor_copy(wo3[:, :, 1], w1[:])
                iout = res_pool.tile([128, NSUB * 2], I32)
                io3 = iout[:].rearrange("p (s k) -> p s k", k=2)
                nc.vector.tensor_copy(io3[:, :, 0], i8[:, :, 0])
                nc.vector.tensor_copy(io3[:, :, 1], idx1f[:])

                nc.sync.dma_start(out_idx[:].rearrange("(s p) k -> p s k", p=128), io3)
                nc.sync.dma_start(out_wt[:].rearrange("(s p) k -> p s k", p=128), wo3)

    nc.compile()
    return nc


_CACHE = {}
LAST_RESULT = None


def _get_nc():
    if "nc" not in _CACHE:
        _CACHE["nc"] = build_module()
    return _CACHE["nc"]


def kernel(hidden_states, weight, noise, _trace=False):
    global LAST_RESULT
    from concourse.bass_utils import run_bass_kernel_spmd

    nc = _get_nc()
    xf = np.ascontiguousarray(np.asarray(hidden_states, dtype=np.float32)).reshape(
        -1, H
    )
    wf = np.ascontiguousarray(np.asarray(weight, dtype=np.float32))
    nf = np.ascontiguousarray(np.asarray(noise, dtype=np.float32))
    in_maps = []
    for c in range(NCORES):
        sl = slice(c * NT, (c + 1) * NT)
        in_maps.append(
            {
                "x": np.ascontiguousarray(xf[sl]),
                "w": wf,
                "noise": np.ascontiguousarray(nf[sl]),
            }
        )
    res = run_bass_kernel_spmd(nc, in_maps, core_ids=list(range(NCORES)), trace=_trace)
    LAST_RESULT = res
    idx = np.concatenate([r["out_idx"] for r in res.results], axis=0).astype(np.int32)
    wts = np.concatenate([r["out_wt"] for r in res.results], axis=0).astype(np.float32)
    return idx, wts
